# revision 3
# baseline (speedup 1.0000x reference)
"""OFA attention (dense_transformer) on 8 Trainium2 NeuronCores.

Sharding: heads split over cores (core c owns heads {2c, 2c+1}, both batches).
Per-core Bass/Tile program (see build_attention_nc below):
  phase 1 : QT/KT/VT = W_c @ hs.T (transposed projections; SCALING folded into Wq,
            c_attn folded into Wv on host; bias-add fused into PSUM drain on ScalarE)
  phase 1b: V natural = PE-transpose(VT), packed [V_A | 1 | V_B | 1] bf16
  phase 2 : per (batch, 512-token t-block), streaming 128-row s-tiles:
              ST(s,t) = K Q^T           (row-tiled K=64 matmuls, 2 heads concurrent)
              ST += bias.T              (PE transposes bias: matmul(lhsT=bias_tile,
                                         rhs=Identity) == bias_tile.T, accumulated
                                         into the scores PSUM -- no elementwise pass)
              E = exp(ST)               (ScalarE, PSUM -> SBUF bf16; exp without
                                         max-subtraction: scores+bias stay in [-8, 8])
              [O.T ; sums] += [V|1].T@E (PV matmul also produces softmax denominators)
            sums transposed to columns by tiny PE matmuls; one wide reciprocal; the
            out-projection runs row-tiled per head and the 1/sums normalization is
            applied at PSUM drain as a per-partition scale (division commutes past
            the per-head d-contraction); heads summed on DVE.
Host: partial outputs summed over cores + bo (the "all-reduce" of the out-projection).
"""
import sys

for _p in ("/opt/trn_rl_repo",):
    if _p not in sys.path:
        sys.path.append(_p)

import numpy as np

import concourse.bass as bass
import concourse.tile as tile
from concourse import mybir
from concourse.masks import make_identity
from concourse.bass_utils import run_bass_kernel_spmd

F32 = mybir.dt.float32
BF16 = mybir.dt.bfloat16

B, T, E, NH, D = 2, 2048, 1024, 16, 64
N_CORES = 8
HPC = NH // N_CORES
DH = HPC * D
SCALING = float(D * 2.0) ** -0.5


def _waitfix(nc, limit=1):
    """This walrus build accepts at most ONE sync-wait per instruction.
    Hoist excess sem-waits onto inserted single-wait NoOps."""
    n_fixed = 0
    for bb in nc.m.functions[0].blocks:
        i = 0
        insts = bb.instructions
        while i < len(insts):
            inst = insts[i]
            si = inst.sync_info
            if si and si.on_wait and len(si.on_wait) > limit:
                extra = si.on_wait[limit:]
                si.on_wait = si.on_wait[:limit]
                for k, w in enumerate(extra):
                    nop = mybir.InstNoOp(
                        name=f"{inst.name}-waitfix{k}",
                        engine=inst.engine,
                        sync_info=mybir.SyncInfo(on_wait=[w], on_update=[]),
                        bass_nofuse=True,
                    )
                    nc.register_instruction(nop, overwrite=True)
                    insts.insert(i, nop)
                    i += 1
                n_fixed += 1
            i += 1
    return n_fixed


def build_attention_nc(B=2, T=2048, E=1024, HPC=2, D=64, with_mask=False,
                       T_BLOCK=512, PROJ_BLOCK=512):
    """Build the per-core Bass program. Returns nc."""
    S = T
    TOK = B * T
    DH = HPC * D                      # 128
    assert DH == 128 and D == 64
    NE = E // 128                     # e-tiles
    NST = S // 128                    # s-tiles per batch
    NTB = T // T_BLOCK                # t-blocks per batch
    NJ = T_BLOCK // 128               # t-subtiles per block
    NPB = TOK // PROJ_BLOCK           # proj token blocks

    nc = bass.Bass()

    hsT = nc.declare_dram_parameter("hsT", [E, TOK], BF16, isOutput=False)
    wqT = nc.declare_dram_parameter("wqT", [E, DH], BF16, isOutput=False)
    wkT = nc.declare_dram_parameter("wkT", [E, DH], BF16, isOutput=False)
    wvT = nc.declare_dram_parameter("wvT", [E, DH], BF16, isOutput=False)
    bq = nc.declare_dram_parameter("bq", [DH, 1], F32, isOutput=False)
    bk = nc.declare_dram_parameter("bk", [DH, 1], F32, isOutput=False)
    bv = nc.declare_dram_parameter("bv", [DH, 1], F32, isOutput=False)
    woT = nc.declare_dram_parameter("woT", [DH, E], BF16, isOutput=False)
    bias_in = nc.declare_dram_parameter("bias", [B, HPC, T, S], F32, isOutput=False)
    if with_mask:
        mask_in = nc.declare_dram_parameter("mask", [B, T, S], F32, isOutput=False)
    out_partial = nc.declare_dram_parameter("out", [TOK, E], BF16, isOutput=True)

    with tile.TileContext(nc) as tc:
        from contextlib import ExitStack
        with ExitStack() as ctx:
            consts = ctx.enter_context(tc.tile_pool(name="consts", bufs=1))
            persist = ctx.enter_context(tc.tile_pool(name="persist", bufs=1))

            i_bf = consts.tile([128, 128], BF16, tag="i_bf")
            make_identity(nc, i_bf[:])
            i_f32 = consts.tile([128, 128], F32, tag="i_f32")
            make_identity(nc, i_f32[:])
            one_sb = consts.tile([1, 1], F32, tag="one_sb")
            nc.vector.memset(one_sb[:], 1.0)

            # weights: (E, DH) -> (128, NE, DH), bf16
            w_sb = {}
            for name, src in (("wq", wqT), ("wk", wkT), ("wv", wvT)):
                t = consts.tile([128, NE, DH], BF16, tag=name)
                nc.sync.dma_start(out=t[:], in_=src.rearrange("(n p) d -> p n d", p=128))
                w_sb[name] = t
            wo_sb = consts.tile([128, E], BF16, tag="wo")
            nc.sync.dma_start(out=wo_sb[:], in_=woT[:, :])
            b_sb = {}
            for name, src in (("bq", bq), ("bk", bk), ("bv", bv)):
                t = consts.tile([128, 1], F32, tag=name)
                nc.sync.dma_start(out=t[:], in_=src[:, :])
                b_sb[name] = t

            # persistent activations (QT/KT bf16; VT f32 for the PE transpose)
            QT = persist.tile([128, TOK], BF16, tag="QT")
            KT = persist.tile([128, TOK], BF16, tag="KT")
            VT = persist.tile([128, TOK], F32, tag="VT")
            V_sb = persist.tile([128, TOK // 128, 2 * (D + 1)], BF16, tag="V_sb")
            nc.vector.memset(V_sb[:, :, D:D + 1], 1.0)
            nc.vector.memset(V_sb[:, :, 2 * D + 1:2 * D + 2], 1.0)

            # ---------------- phase 1: projections ----------------
            with tc.tile_pool(name="hst", bufs=16) as hst_pool, \
                 tc.tile_pool(name="proj_ps", bufs=3, space="PSUM") as proj_ps:
                for pb in range(NPB):
                    t0 = pb * PROJ_BLOCK
                    hst = []
                    for e in range(NE):
                        h = hst_pool.tile([128, PROJ_BLOCK], BF16, tag="hst",
                                          name=f"hst{pb}_{e}")
                        nc.sync.dma_start(
                            out=h[:], in_=hsT[e * 128:(e + 1) * 128, t0:t0 + PROJ_BLOCK])
                        hst.append(h)
                    for name, dst in (("wq", QT), ("wk", KT), ("wv", VT)):
                        ps = proj_ps.tile([128, PROJ_BLOCK], F32, tag="proj",
                                          name=f"pps{pb}_{name}")
                        for e in range(NE):
                            nc.tensor.matmul(ps[:], w_sb[name][:, e, :], hst[e][:],
                                             start=(e == 0), stop=(e == NE - 1))
                        nc.scalar.activation(
                            out=dst[:, t0:t0 + PROJ_BLOCK], in_=ps[:],
                            func=mybir.ActivationFunctionType.Identity,
                            bias=b_sb["b" + name[1]][:], scale=1.0)

            # ---------------- phase 1b: V natural ----------------
            with tc.tile_pool(name="vtr_ps", bufs=2, space="PSUM") as vtr_ps:
                for st in range(TOK // 128):
                    ps = vtr_ps.tile([128, 128], F32, tag="vtr", name=f"vtr{st}")
                    nc.tensor.transpose(ps[:], VT[:, st * 128:(st + 1) * 128], i_f32[:])
                    nc.vector.tensor_copy(out=V_sb[:, st, 0:D], in_=ps[:, 0:D])
                    nc.vector.tensor_copy(out=V_sb[:, st, D + 1:2 * D + 1],
                                          in_=ps[:, D:2 * D])

            # ---------------- phase 2: attention ----------------
            with tc.tile_pool(name="bias_sb", bufs=3 * HPC * 2, space="SBUF") as bias_pool, \
                 tc.tile_pool(name="mask_sb", bufs=3, space="SBUF") as mask_pool, \
                 tc.tile_pool(name="e_sb", bufs=4) as e_pool, \
                 tc.tile_pool(name="ot_sb", bufs=2) as ot_sb_pool, \
                 tc.tile_pool(name="sums", bufs=4) as sums_pool, \
                 tc.tile_pool(name="rcol", bufs=2) as rcol_pool, \
                 tc.tile_pool(name="tmp", bufs=3) as tmp_pool, \
                 tc.tile_pool(name="osb", bufs=3) as out_pool, \
                 tc.tile_pool(name="st_ps", bufs=4, space="PSUM") as st_ps, \
                 tc.tile_pool(name="ot_ps", bufs=2, space="PSUM") as ot_ps, \
                 tc.tile_pool(name="wo_ps", bufs=2, space="PSUM") as wo_ps:
                for b in range(B):
                    for tb in range(NTB):
                        tglob = b * T + tb * T_BLOCK
                        bs = []
                        for a in range(HPC):
                            row = []
                            for j in range(NJ):
                                t = bias_pool.tile([128, S], BF16, tag="bias",
                                                   name=f"bias{b}_{tb}_{a}_{j}")
                                tr = tb * T_BLOCK + j * 128
                                nc.gpsimd.dma_start(
                                    out=t[:], in_=bias_in[b, a, tr:tr + 128, :])
                                row.append(t)
                            bs.append(row)
                        if with_mask:
                            ms = []
                            for j in range(NJ):
                                t = mask_pool.tile([128, S], BF16, tag="mask",
                                                   name=f"mask{b}_{tb}_{j}")
                                tr = tb * T_BLOCK + j * 128
                                nc.gpsimd.dma_start(
                                    out=t[:], in_=mask_in[b, tr:tr + 128, :])
                                ms.append(t)

                        ots = [ot_ps.tile([D + 1, T_BLOCK], F32, tag="ot",
                                          name=f"ot{b}_{tb}_{a}") for a in range(HPC)]
                        for st in range(NST):
                            sg = b * T + st * 128
                            stps = []
                            for a in range(HPC):
                                stp = st_ps.tile([128, T_BLOCK], F32, tag="st",
                                                 name=f"st{b}_{tb}_{st}_{a}")
                                r0 = a * D
                                nc.tensor.matmul(
                                    stp[:],
                                    KT[r0:r0 + D, sg:sg + 128],
                                    QT[r0:r0 + D, tglob:tglob + T_BLOCK],
                                    start=True, stop=False)
                                stps.append(stp)
                            for a in range(HPC):
                                stp = stps[a]
                                for j in range(NJ):
                                    nc.tensor.matmul(
                                        stp[:, j * 128:(j + 1) * 128],
                                        bs[a][j][:, st * 128:(st + 1) * 128],
                                        i_bf[:],
                                        start=False,
                                        stop=(j == NJ - 1 and not with_mask))
                                if with_mask:
                                    for j in range(NJ):
                                        nc.tensor.matmul(
                                            stp[:, j * 128:(j + 1) * 128],
                                            ms[j][:, st * 128:(st + 1) * 128],
                                            i_bf[:],
                                            start=False, stop=(j == NJ - 1))
                                e_t = e_pool.tile([128, T_BLOCK], BF16, tag="et",
                                                  name=f"et{b}_{tb}_{st}_{a}")
                                nc.scalar.activation(
                                    out=e_t[:], in_=stp[:],
                                    func=mybir.ActivationFunctionType.Exp)
                                nc.tensor.matmul(
                                    ots[a][:],
                                    V_sb[:, (b * T + st * 128) // 128,
                                         a * (D + 1):(a + 1) * (D + 1)],
                                    e_t[:],
                                    start=(st == 0), stop=(st == NST - 1))

                        # drain O.T (bf16) + sums rows; frees ot psum quickly
                        otn = ot_sb_pool.tile([128, T_BLOCK], BF16, tag="otn",
                                              name=f"otn{b}_{tb}")
                        sums_sb = []
                        for a in range(HPC):
                            nc.vector.tensor_copy(out=otn[a * D:(a + 1) * D, :],
                                                  in_=ots[a][0:D, :])
                            ss = sums_pool.tile([1, T_BLOCK], F32, tag="sums",
                                                name=f"sums{b}_{tb}_{a}")
                            nc.vector.tensor_copy(out=ss[:], in_=ots[a][D:D + 1, :])
                            sums_sb.append(ss)
                        # transpose sums into columns on the PE, then one wide recip
                        rps = st_ps.tile([128, HPC * NJ], F32, tag="st",
                                          name=f"nm{b}_{tb}")
                        nmm = 0
                        for a in range(HPC):
                            for k in range(NJ):
                                nc.tensor.matmul(
                                    rps[:, a * NJ + k:a * NJ + k + 1],
                                    sums_sb[a][0:1, k * 128:(k + 1) * 128],
                                    one_sb[:],
                                    start=(nmm == 0), stop=(nmm == HPC * NJ - 1))
                                nmm += 1
                        rcol = rcol_pool.tile([128, HPC * NJ], F32, tag="rcol",
                                              name=f"rcol{b}_{tb}")
                        nc.vector.reciprocal(rcol[:], rps[:])

                        # out projection, row-tiled per head; normalization at drain
                        for k in range(NJ):
                            os_t = out_pool.tile([128, E], BF16, tag="osb",
                                                 name=f"osb{b}_{tb}_{k}")
                            for n0 in range(0, E, 512):
                                nn_ = min(512, E - n0)
                                wpa = wo_ps.tile([128, 512], F32, tag="wo",
                                                 name=f"wopa{b}_{tb}_{k}_{n0}")
                                wpb = wo_ps.tile([128, 512], F32, tag="wo",
                                                 name=f"wopb{b}_{tb}_{k}_{n0}")
                                nc.tensor.matmul(
                                    wpa[:, 0:nn_],
                                    otn[0:D, k * 128:(k + 1) * 128],
                                    wo_sb[0:D, n0:n0 + nn_],
                                    start=True, stop=True)
                                nc.tensor.matmul(
                                    wpb[:, 0:nn_],
                                    otn[D:2 * D, k * 128:(k + 1) * 128],
                                    wo_sb[D:2 * D, n0:n0 + nn_],
                                    start=True, stop=True)
                                tmp = tmp_pool.tile([128, 512], F32, tag="tmp",
                                                    name=f"tmp{b}_{tb}_{k}_{n0}")
                                nc.scalar.activation(
                                    out=tmp[:, 0:nn_], in_=wpa[:, 0:nn_],
                                    func=mybir.ActivationFunctionType.Copy,
                                    scale=rcol[:, 0 * NJ + k:0 * NJ + k + 1])
                                nc.vector.tensor_scalar_mul(
                                    out=os_t[:, n0:n0 + nn_], in0=wpb[:, 0:nn_],
                                    scalar1=rcol[:, 1 * NJ + k:1 * NJ + k + 1])
                                nc.vector.tensor_add(
                                    out=os_t[:, n0:n0 + nn_],
                                    in0=os_t[:, n0:n0 + nn_], in1=tmp[:, 0:nn_])
                            nc.sync.dma_start(
                                out=out_partial[tglob + k * 128: tglob + (k + 1) * 128, :],
                                in_=os_t[:])
    _waitfix(nc)
    return nc


# ---------------- host-side prep ----------------

def shard_inputs(hidden_states, attn_bias, attention_mask, Wq, bq, Wk, bk, Wv, bv,
                 Wo, bo, c_attn, n_cores=8, scaling=None):
    """Build per-core input maps. Returns (in_maps, with_mask)."""
    import ml_dtypes
    bf16 = ml_dtypes.bfloat16
    B, T, E = hidden_states.shape
    NH = c_attn.shape[0]
    D = E // NH
    HPC = NH // n_cores
    DH = HPC * D

    with_mask = bool(np.any(attention_mask))
    hsT = np.ascontiguousarray(hidden_states.reshape(B * T, E).T).astype(bf16)
    bias4 = attn_bias.reshape(B, NH, T, T)

    if scaling is None:
        scaling = float(D * 2.0) ** -0.5

    in_maps = []
    for c in range(n_cores):
        r0 = c * DH
        sl = slice(r0, r0 + DH)
        cvec = np.repeat(c_attn[c * HPC:(c + 1) * HPC], D)
        m = {
            "hsT": hsT,
            "wqT": np.ascontiguousarray((Wq[sl] * scaling).T).astype(bf16),
            "wkT": np.ascontiguousarray(Wk[sl].T).astype(bf16),
            "wvT": np.ascontiguousarray((Wv[sl] * cvec[:, None]).T).astype(bf16),
            "bq": np.ascontiguousarray((bq[sl] * scaling)[:, None]).astype(np.float32),
            "bk": np.ascontiguousarray(bk[sl][:, None]).astype(np.float32),
            "bv": np.ascontiguousarray((bv[sl] * cvec)[:, None]).astype(np.float32),
            "woT": np.ascontiguousarray(Wo[:, sl].T).astype(bf16),
            "bias": np.ascontiguousarray(bias4[:, c * HPC:(c + 1) * HPC]),
        }
        if with_mask:
            m["mask"] = np.ascontiguousarray(
                np.broadcast_to(attention_mask.reshape(B, 1, T, T)[:, 0],
                                (B, T, T))).astype(np.float32)
        in_maps.append(m)
    return in_maps, with_mask




_NC_CACHE = {}


def run_spmd(in_maps, with_mask, **kwargs):
    if with_mask not in _NC_CACHE:
        _NC_CACHE[with_mask] = build_attention_nc(
            B=B, T=T, E=E, HPC=HPC, D=D, with_mask=with_mask)
    nc = _NC_CACHE[with_mask]
    return run_bass_kernel_spmd(nc, in_maps, list(range(N_CORES)), **kwargs)


def kernel(hidden_states, attn_bias, attention_mask, Wq, bq, Wk, bk, Wv, bv,
           Wo, bo, c_attn):
    args = [np.asarray(a, dtype=np.float32) for a in
            (hidden_states, attn_bias, attention_mask, Wq, bq, Wk, bk, Wv, bv,
             Wo, bo, c_attn)]
    (hidden_states, attn_bias, attention_mask, Wq, bq, Wk, bk, Wv, bv,
     Wo, bo, c_attn) = args
    in_maps, with_mask = shard_inputs(hidden_states, attn_bias, attention_mask,
                                      Wq, bq, Wk, bk, Wv, bv, Wo, bo, c_attn,
                                      n_cores=N_CORES, scaling=SCALING)
    res = run_spmd(in_maps, with_mask)
    out = np.zeros((B * T, E), np.float32)
    for r in res.results:
        out += r["out"]
    out += bo[None, :]
    return out.reshape(B, T, E).astype(np.float32)
